# revision 1
# baseline (speedup 1.0000x reference)
"""Trainium2 Bass kernel for CG-SENSE MRI reconstruction (nn_CGClass).

Problem: for each of B=4 samples solve M x = rhs by 10 CG iterations where
  M(p)  = sum_c conj(s_c) * ifft2(mask * fft2(s_c * p)) + lam * p
  rhs   = sum_c conj(s_c) * ifft2(mask * y_c) + lam * x_in
(all ffts norm='ortho', images 384x384, C=16 coils).

Implementation notes:
- fft2 is computed with zero transposes via the identity
  P2(U) = U^T @ F  (tensor engine computes lhsT.T @ rhs, so feeding U as
  lhsT gives the transpose for free);  P2(P2(U)) = F U F = fft2(U) since the
  DFT matrix F is symmetric.  ifft2 uses conj(F).
- Complex matmuls: 4 real matmuls accumulated pairwise in PSUM using a
  precomputed negated imaginary DFT matrix (no vector-engine combines).
- CG updates are PE-free: cross-partition dot reduction uses the Pool
  engine's partition_all_reduce into [P,1]-replicated scalars, lam*p is
  folded into the Ap partials before the AllReduce (lam/4 per core), and
  the per-partition p.(Ap+lam p) partials ride inside the AllReduce payload
  so no big dot sits on the post-collective critical path.
- The last CG iteration only needs alpha = rTr/p^T M p, and p^T M p =
  ||mask*fft2(s_c p)||^2 + lam||p||^2, so iteration 10 runs just the two
  forward stages per coil plus a squared-norm reduce, and its AllReduce
  payload is [P,1].
- Sharding: 8 cores = 2 groups of 4. Group g owns samples (2g, 2g+1); each
  core holds 4 of the 16 coils for both samples. Per CG iteration each core
  computes its 4-coil partial of M(p) for each sample; partials are summed
  with a 4-rank AllReduce per sample. The two samples' solves interleave so
  collectives overlap the other sample's compute. All cores of a group end
  with identical CG state; the host reads cores 0 and 4.
"""

import os
import sys
import types

import numpy as np

import concourse.bacc as bacc
import concourse.mybir as mybir
import concourse.tile as tile
import concourse.bass_isa as bass_isa
from concourse.bass_utils import run_bass_kernel_spmd

P = 128          # SBUF partitions
N = 384          # image side
NT = 3           # partition tiles per image side (3*128 = 384)
PSW = 512        # psum bank width in f32
F32 = mybir.dt.float32
F32R = mybir.dt.float32r
ADD = mybir.AluOpType.add
SUB = mybir.AluOpType.subtract
MUL = mybir.AluOpType.mult

USE_F32R = True   # reduced-precision single-pass PE mode (4x faster than fp32)


# ----------------------------------------------------------------------------
# host-side layout helpers (pure data movement)
# ----------------------------------------------------------------------------

def _to_tiles(img):
    """[384, X...] -> [128, 3, X...] partition-tiled layout."""
    return np.ascontiguousarray(
        img.reshape(NT, P, *img.shape[1:]).transpose(1, 0, *range(2, img.ndim + 1))
    )


def _from_tiles(t):
    """[128, 3, X] -> [384, X]."""
    return np.ascontiguousarray(t.transpose(1, 0, 2)).reshape(N, t.shape[-1])


def _complex_tiles(re, im):
    """two [384,384] -> [128, 2, 3, 384]"""
    return np.ascontiguousarray(
        np.stack([_to_tiles(re), _to_tiles(im)], axis=1))


def _fmats():
    k = np.arange(N)
    Fm = np.exp(-2j * np.pi * np.outer(k, k) / N) / np.sqrt(N)
    fr = _to_tiles(Fm.real.astype(np.float32))
    fi = _to_tiles(Fm.imag.astype(np.float32))
    return np.ascontiguousarray(np.stack([fr, fi, -fi]))  # [3, 128, 3, 384]


# ----------------------------------------------------------------------------
# kernel builder
# ----------------------------------------------------------------------------

def build_cg(lam, n_iters, cpc, n_samples, group_size, use_f32r, n_cores):
    """Build the SPMD program (one program, data-parallel across cores).

    cpc: coils per core (per sample); full coil count = cpc * group_size.
    n_samples: samples per group (interleaved CG solves).
    """
    MMDT = F32R if use_f32r else F32
    nc = bacc.Bacc("TRN2", target_bir_lowering=False, debug=False,
                   num_devices=n_cores)

    n_groups = n_cores // group_size
    groups = [[g * group_size + j for j in range(group_size)]
              for g in range(n_groups)]
    use_ar = group_size > 1

    smaps_d = nc.dram_tensor("smaps", [n_samples, cpc, P, 2, NT, N], F32,
                             kind="ExternalInput")
    y_d = nc.dram_tensor("y", [n_samples, cpc, P, 2, NT, N], F32,
                         kind="ExternalInput")
    mask_d = nc.dram_tensor("mask", [n_samples, P, NT, N], F32,
                            kind="ExternalInput")
    xin_d = nc.dram_tensor("xin", [n_samples, P, 2, NT, N], F32,
                           kind="ExternalInput")
    fmat_d = nc.dram_tensor("fmat", [3, P, NT, N], F32, kind="ExternalInput")
    out_d = nc.dram_tensor("out", [n_samples, P, 2, NT, N], F32,
                           kind="ExternalOutput")

    with tile.TileContext(nc) as tc:
        with (
            tc.tile_pool(name="const", bufs=1) as cpool,
            tc.tile_pool(name="cg", bufs=1) as cgpool,
            tc.tile_pool(name="stage", bufs=5) as stpool,
            tc.tile_pool(name="smap", bufs=3) as smpool,
            tc.tile_pool(name="tmp", bufs=2) as tmppool,
            tc.tile_pool(name="x4", bufs=1) as x4pool,
            tc.tile_pool(name="ac", bufs=1) as acpool,
            tc.tile_pool(name="scal", bufs=6) as scpool,
            tc.tile_pool(name="ps", bufs=8, space="PSUM") as pspool,
            tc.tile_pool(name="dram", bufs=4, space="DRAM") as drpool,
        ):
            # ---- constants ----
            f_sb = cpool.tile([P, 3, NT, N], MMDT, tag="F")
            nc.gpsimd.dma_start(f_sb[:], fmat_d[:].rearrange("m p t n -> p m t n"))
            FR, FI, FNI = f_sb[:, 0], f_sb[:, 1], f_sb[:, 2]
            # forward fft rhs parts: (re=FR, im=FI, negim=FNI)
            # inverse fft rhs parts: (re=FR, im=FNI, negim=FI)

            mask_sb = []
            for s in range(n_samples):
                m = cpool.tile([P, NT, N], F32, tag=f"mask{s}", name=f"mask{s}")
                nc.sync.dma_start(m[:], mask_d[s])
                mask_sb.append(m)

            # ---- persistent CG state ----
            xs, rs, ps_, aps = [], [], [], []
            for s in range(n_samples):
                xs.append(cgpool.tile([P, 2, NT, N], F32, tag=f"x{s}", name=f"x{s}"))
                rs.append(cgpool.tile([P, 2, NT, N], F32, tag=f"r{s}", name=f"r{s}"))
                ps_.append(cgpool.tile([P, 2, NT, N], F32, tag=f"p{s}", name=f"p{s}"))
                aps.append(cgpool.tile([P, 2, NT, N], F32, tag=f"ap{s}", name=f"ap{s}"))
            dsums = [cgpool.tile([P, 1], F32, tag=f"dsum{s}", name=f"dsum{s}")
                     for s in range(n_samples)]
            # ---------------- helpers ----------------
            def p2_mm_mtile(src, rhs_parts, m, pr, pi):
                """12 matmuls producing output m-tile (re+im) of one complex
                P2 stage into single-bank psum tiles pr/pi [P, PSW]."""
                R, I, NI = rhs_parts
                ms = slice(m * P, (m + 1) * P)
                # weight-major pairing: each lhsT tile feeds two consecutive
                # matmuls (re and im outputs) so the weight load amortizes
                for k in range(NT):
                    nc.tensor.matmul(pr[:, 0:N], src[:, 0, k, ms],
                                     R[:, k, :], start=(k == 0), stop=False)
                    nc.tensor.matmul(pi[:, 0:N], src[:, 0, k, ms],
                                     I[:, k, :], start=(k == 0), stop=False)
                    nc.tensor.matmul(pr[:, 0:N], src[:, 1, k, ms],
                                     NI[:, k, :], start=False,
                                     stop=(k == NT - 1))
                    nc.tensor.matmul(pi[:, 0:N], src[:, 1, k, ms],
                                     R[:, k, :], start=False,
                                     stop=(k == NT - 1))

            def p2_plain(src, rhs_parts, dst):
                """dst = P2(src), dst an MMDT [P,2,NT,N] tile (ACT evacuation,
                per m-tile so next-stage matmuls can start after 1/3)."""
                for m in range(NT):
                    pr = pspool.tile([P, PSW], F32, tag="ps")
                    pi = pspool.tile([P, PSW], F32, tag="ps")
                    p2_mm_mtile(src, rhs_parts, m, pr, pi)
                    nc.scalar.copy(dst[:, 0, m], pr[:, 0:N])
                    nc.scalar.copy(dst[:, 1, m], pi[:, 0:N])

            def p2_mask_f32(src, rhs_parts, dst, msk):
                """like p2_mask but into an f32 [P,2,NT,N] tile."""
                for m in range(NT):
                    pr = pspool.tile([P, PSW], F32, tag="ps")
                    pi = pspool.tile([P, PSW], F32, tag="ps")
                    p2_mm_mtile(src, rhs_parts, m, pr, pi)
                    nc.vector.tensor_tensor(dst[:, 0, m], pr[:, 0:N],
                                            msk[:, m], op=MUL)
                    nc.vector.tensor_tensor(dst[:, 1, m], pi[:, 0:N],
                                            msk[:, m], op=MUL)

            def p2_mask(src, rhs_parts, dst, msk):
                """dst = P2(src) * mask (fused into PSUM evacuation)."""
                for m in range(NT):
                    pr = pspool.tile([P, PSW], F32, tag="ps")
                    pi = pspool.tile([P, PSW], F32, tag="ps")
                    p2_mm_mtile(src, rhs_parts, m, pr, pi)
                    nc.vector.tensor_tensor(dst[:, 0, m], pr[:, 0:N],
                                            msk[:, m], op=MUL)
                    nc.vector.tensor_tensor(dst[:, 1, m], pi[:, 0:N],
                                            msk[:, m], op=MUL)

            def p2_accum(src, rhs_parts, smap, acc, first, lam_seed=None):
                """acc (+)= conj(smap) * P2(src)   [the final ifft stage].

                PSUM is drained by cheap ACT copies into x4; the complex
                multiply-accumulate runs as whole-image f32 DVE ops."""
                x4 = x4pool.tile([P, 2, NT, N], F32, tag="x4")
                for m in range(NT):
                    pr = pspool.tile([P, PSW], F32, tag="ps")
                    pi = pspool.tile([P, PSW], F32, tag="ps")
                    p2_mm_mtile(src, rhs_parts, m, pr, pi)
                    nc.scalar.copy(x4[:, 0, m], pr[:, 0:N])
                    nc.scalar.copy(x4[:, 1, m], pi[:, 0:N])
                ac = acpool.tile([P, 2, NT, N], F32, tag="ac")
                t0, t1 = ac[:, 0], ac[:, 1]
                if first and lam_seed is not None:
                    # seed acc = (lam/group_size) * p here, off the
                    # end-of-chain critical path
                    nc.vector.tensor_scalar_mul(
                        acc[:], lam_seed[:], float(lam) / group_size)
                nc.vector.tensor_tensor(t0, x4[:, 0], smap[:, 0], op=MUL)
                nc.vector.tensor_tensor(t1, x4[:, 1], smap[:, 1], op=MUL)
                if first and lam_seed is None:
                    nc.vector.tensor_tensor(acc[:, 0], t0, t1, op=ADD)
                else:
                    nc.vector.tensor_tensor(acc[:, 0], acc[:, 0], t0, op=ADD)
                    nc.vector.tensor_tensor(acc[:, 0], acc[:, 0], t1, op=ADD)
                nc.vector.tensor_tensor(t0, x4[:, 1], smap[:, 0], op=MUL)
                nc.vector.tensor_tensor(t1, x4[:, 0], smap[:, 1], op=MUL)
                if first and lam_seed is None:
                    nc.vector.tensor_tensor(acc[:, 1], t0, t1, op=SUB)
                else:
                    nc.vector.tensor_tensor(acc[:, 1], acc[:, 1], t0, op=ADD)
                    nc.vector.tensor_tensor(acc[:, 1], acc[:, 1], t1, op=SUB)

            FWD = (FR, FI, FNI)
            INV = (FR, FNI, FI)

            def make_q(s, smap):
                """q = smap * p_s (complex front multiply; software-pipelined
                one coil ahead so the DVE computes it during the previous
                chain's matmul phases)."""
                p = ps_[s]
                q = stpool.tile([P, 2, NT, N], MMDT, tag="st")
                t1 = tmppool.tile([P, NT, N], F32, tag="ft1", bufs=1)
                nc.vector.tensor_tensor(q[:, 0], smap[:, 0], p[:, 0], op=MUL)
                nc.vector.tensor_tensor(t1[:], smap[:, 1], p[:, 1], op=MUL)
                nc.vector.tensor_tensor(q[:, 0], q[:, 0], t1[:], op=SUB)
                t2 = tmppool.tile([P, NT, N], F32, tag="ft2", bufs=1)
                nc.vector.tensor_tensor(q[:, 1], smap[:, 0], p[:, 1], op=MUL)
                nc.vector.tensor_tensor(t2[:], smap[:, 1], p[:, 0], op=MUL)
                nc.vector.tensor_tensor(q[:, 1], q[:, 1], t2[:], op=ADD)
                return q

            def chain_rest(s, q, smap, first, lam_seed=None):
                """fft2 -> mask -> ifft2 -> conj(smap) accumulate for one coil."""
                x1 = stpool.tile([P, 2, NT, N], MMDT, tag="st")
                p2_plain(q, FWD, x1)
                x2 = stpool.tile([P, 2, NT, N], MMDT, tag="st")
                p2_mask(x1, FWD, x2, mask_sb[s])
                x4 = stpool.tile([P, 2, NT, N], MMDT, tag="st")
                p2_plain(x2, INV, x4)
                p2_accum(x4, INV, smap, aps[s], first, lam_seed=lam_seed)

            def load_smap(s, c):
                t = smpool.tile([P, 2, NT, N], F32, tag="sm")
                nc.sync.dma_start(t[:], smaps_d[s, c])
                return t

            NF = 2 * NT * N

            def allreduce(acc, dd=None):
                """AllReduce acc [P,2,NT,N]; the [P,1] dot partials dd
                ride inside the same payload."""
                if not use_ar:
                    if dd is not None:
                        nc.vector.tensor_copy(dd[1][:], dd[0][:])
                    return
                w = NF + (1 if dd is not None else 0)
                bi = drpool.tile([P, w], F32, tag=f"bi{w}")
                bo = drpool.tile([P, w], F32, tag=f"bo{w}")
                nc.sync.dma_start(bi[:, 0:NF],
                                  acc[:].rearrange("p a t n -> p (a t n)"))
                if dd is not None:
                    nc.sync.dma_start(bi[:, NF:NF + 1], dd[0][:])
                nc.gpsimd.collective_compute(
                    "AllReduce", ADD, replica_groups=groups,
                    ins=[bi[:].opt()], outs=[bo[:].opt()])
                nc.sync.dma_start(acc[:].rearrange("p a t n -> p (a t n)"),
                                  bo[:, 0:NF])
                if dd is not None:
                    nc.sync.dma_start(dd[1][:], bo[:, NF:NF + 1])

            def allreduce_small(dd_in, dd_out):
                """AllReduce just a [P,1] vector (last-iteration dot)."""
                if not use_ar:
                    nc.vector.tensor_copy(dd_out[:], dd_in[:])
                    return
                bi = drpool.tile([P, 1], F32, tag="sbi")
                bo = drpool.tile([P, 1], F32, tag="sbo")
                nc.sync.dma_start(bi[:], dd_in[:])
                nc.gpsimd.collective_compute(
                    "AllReduce", ADD, replica_groups=groups,
                    ins=[bi[:].opt()], outs=[bo[:].opt()])
                nc.sync.dma_start(dd_out[:], bo[:])

            def dot_partials(a, b):
                """per-partition partial sums of a*b -> [P,1] f32.

                (tensor_tensor_reduce miscompiles on HW; use mult+reduce.)"""
                ppa = scpool.tile([P, 1], F32, tag="ppa")
                ppb = scpool.tile([P, 1], F32, tag="ppb")
                ta = scpool.tile([P, NT, N], F32, tag="dsa", bufs=1)
                nc.vector.tensor_tensor(ta[:], a[:, 0], b[:, 0], op=MUL)
                nc.vector.tensor_reduce(ppa[:], ta[:],
                                        axis=mybir.AxisListType.XY, op=ADD)
                tb = scpool.tile([P, NT, N], F32, tag="dsb", bufs=1)
                nc.vector.tensor_tensor(tb[:], a[:, 1], b[:, 1], op=MUL)
                nc.vector.tensor_reduce(ppb[:], tb[:],
                                        axis=mybir.AxisListType.XY, op=ADD)
                pp = scpool.tile([P, 1], F32, tag="pp")
                nc.vector.tensor_tensor(pp[:], ppa[:], ppb[:], op=ADD)
                return pp

            def preduce(pp):
                """[P,1] partials -> [P,1] replicated total (Pool, no PE)."""
                out = scpool.tile([P, 1], F32, tag="prs")
                nc.gpsimd.partition_all_reduce(out[:], pp[:], 128,
                                               bass_isa.ReduceOp.add)
                return out

            def dot_all(a, b):
                """sum(a*b) -> [P,1] replicated (no PE involvement)."""
                return preduce(dot_partials(a, b))

            # ---------------- rhs phase ----------------
            # aps[s] <- partial AH(y) ; AR ; p = r = rhs = aps + lam*xin; x = 0
            rtr = [None] * n_samples
            def make_ym(s, c):
                yt = stpool.tile([P, 2, NT, N], F32, tag="st")
                nc.sync.dma_start(yt[:], y_d[s, c])
                ym = stpool.tile([P, 2, NT, N], MMDT, tag="st")
                nc.vector.tensor_tensor(ym[:, 0], yt[:, 0], mask_sb[s][:], op=MUL)
                nc.vector.tensor_tensor(ym[:, 1], yt[:, 1], mask_sb[s][:], op=MUL)
                return ym

            def rhs_setup(s):
                xin = stpool.tile([P, 2, NT, N], F32, tag="st", name=f"xin{s}")
                nc.sync.dma_start(xin[:], xin_d[s])
                # p = rhs = aps + lam*xin
                nc.vector.scalar_tensor_tensor(
                    out=ps_[s][:], in0=xin[:], scalar=float(lam), in1=aps[s][:],
                    op0=MUL, op1=ADD)
                nc.gpsimd.tensor_copy(rs[s][:], ps_[s][:])
                nc.vector.memset(xs[s][:], 0.0)
                rtr[s] = dot_all(ps_[s], ps_[s])

            def rhs_chains(s, pre_last=None):
                sm = load_smap(s, 0)
                ym = make_ym(s, 0)
                for c in range(cpc):
                    if c + 1 < cpc:
                        sm_n = load_smap(s, c + 1)
                        ym_n = make_ym(s, c + 1)
                    if pre_last is not None and c == cpc - 1:
                        pre_last()
                    w1 = stpool.tile([P, 2, NT, N], MMDT, tag="st")
                    p2_plain(ym, INV, w1)
                    p2_accum(w1, INV, sm, aps[s], first=(c == 0))
                    if c + 1 < cpc:
                        sm, ym = sm_n, ym_n
                allreduce(aps[s])

            rhs_chains(0)

            rhs_chains(1, pre_last=lambda: rhs_setup(0))
            rhs_setup(1)

            # ---------------- CG iterations ----------------
            def cg_update(s, last=False):
                """PE-free CG update: aps[s] already holds AR(Ap + lam p)
                and dsums[s] the AR'd p.(Ap+lam p) per-partition partials.
                last=True: only alpha and x, then stream the output out."""
                pap = preduce(dsums[s])
                ipap = scpool.tile([P, 1], F32, tag="ipap")
                nc.vector.reciprocal(ipap[:], pap[:])
                alpha = scpool.tile([P, 1], F32, tag="alpha")
                nc.vector.tensor_tensor(alpha[:], rtr[s][:], ipap[:], op=MUL)
                # x += alpha p
                nc.vector.scalar_tensor_tensor(
                    out=xs[s][:], in0=ps_[s][:], scalar=alpha[:], in1=xs[s][:],
                    op0=MUL, op1=ADD)
                if last:
                    nc.sync.dma_start(out_d[s], xs[s][:])
                    return
                nab = scpool.tile([P, 1], F32, tag="nab")
                nc.scalar.mul(nab[:], alpha[:], -1.0)
                # r -= alpha (Ap + lam p)
                nc.vector.scalar_tensor_tensor(
                    out=rs[s][:], in0=aps[s][:], scalar=nab[:], in1=rs[s][:],
                    op0=MUL, op1=ADD)
                rtrn = dot_all(rs[s], rs[s])
                irtr = scpool.tile([P, 1], F32, tag="irtr")
                nc.vector.reciprocal(irtr[:], rtr[s][:])
                beta = scpool.tile([P, 1], F32, tag="beta")
                nc.vector.tensor_tensor(beta[:], rtrn[:], irtr[:], op=MUL)
                # p = r + beta p
                nc.vector.scalar_tensor_tensor(
                    out=ps_[s][:], in0=ps_[s][:], scalar=beta[:], in1=rs[s][:],
                    op0=MUL, op1=ADD)
                rtr[s] = rtrn

            def chains(s, pre_last=None):
                sm = load_smap(s, 0)
                q = make_q(s, sm)
                for c in range(cpc):
                    if c + 1 < cpc:
                        sm_n = load_smap(s, c + 1)
                        q_n = make_q(s, sm_n)
                    if pre_last is not None and c == cpc - 1:
                        # emit the other sample's CG update (and the next
                        # sample's first-coil prep) so the serial DVE tail
                        # and the junction make_q hide under the last coil
                        pre_last()
                    chain_rest(s, q, sm, first=(c == 0),
                               lam_seed=(ps_[s] if c == 0 else None))
                    if c + 1 < cpc:
                        sm, q = sm_n, q_n
                # aps already includes (lam/group_size) p from the first-coil
                # seed; ride the p.(Ap+lam p) partials inside the AllReduce
                dd = dot_partials(ps_[s], aps[s])
                allreduce(aps[s], dd=(dd, dsums[s]))

            def chains_fwd_norm(s, pre_last=None):
                """Last iteration: p^T M p = sum_c ||mask*fft2(s_c p)||^2
                + lam ||p||^2 -- forward stages + squared-norm only, with a
                [P,1] AllReduce."""
                nr = scpool.tile([P, 1], F32, tag="nr", name=f"nr{s}")
                sm = load_smap(s, 0)
                q = make_q(s, sm)
                for c in range(cpc):
                    if c + 1 < cpc:
                        sm_n = load_smap(s, c + 1)
                        q_n = make_q(s, sm_n)
                    if pre_last is not None and c == cpc - 1:
                        pre_last()
                    x1 = stpool.tile([P, 2, NT, N], MMDT, tag="st")
                    p2_plain(q, FWD, x1)
                    x2 = x4pool.tile([P, 2, NT, N], F32, tag="x4", name=f"n2{c}")
                    p2_mask_f32(x1, FWD, x2, mask_sb[s])
                    dd = dot_partials(x2, x2)
                    if c == 0:
                        nc.vector.tensor_copy(nr[:], dd[:])
                    else:
                        nc.vector.tensor_tensor(nr[:], nr[:], dd[:], op=ADD)
                    if c + 1 < cpc:
                        sm, q = sm_n, q_n
                # + (lam/group_size) ||p||^2 per core
                pp = dot_partials(ps_[s], ps_[s])
                nc.vector.scalar_tensor_tensor(
                    out=nr[:], in0=pp[:], scalar=float(lam) / group_size,
                    in1=nr[:], op0=MUL, op1=ADD)
                allreduce_small(nr, dsums[s])

            # Software-pipelined schedule: each sample's update is emitted in
            # the middle of the other sample's chain phase of the next
            # iteration, so the update's serial DVE tail (and the AllReduce it
            # waits on) always overlaps matmul work and the PE never drains at
            # iteration boundaries. The final iteration is forward-only.
            if n_samples == 2:
                for it in range(n_iters - 1):
                    chains(0, pre_last=(lambda: cg_update(1)) if it > 0
                           else None)
                    chains(1, pre_last=lambda: cg_update(0))
                chains_fwd_norm(0, pre_last=lambda: cg_update(1))
                chains_fwd_norm(1, pre_last=lambda: cg_update(0, last=True))
                cg_update(1, last=True)
            else:
                for it in range(n_iters - 1):
                    for s in range(n_samples):
                        chains(s)
                    for s in range(n_samples):
                        cg_update(s)
                for s in range(n_samples):
                    chains_fwd_norm(s)
                for s in range(n_samples):
                    cg_update(s, last=True)

    nc.compile()
    return nc


# ----------------------------------------------------------------------------
# public entry point
# ----------------------------------------------------------------------------

_CACHE = {}
LAST_EXEC_NS = None


def _install_ntff_hook():
    """Optional NTFF profiling under axon (dev only; grading runs skip it)."""
    try:
        from trn_agent_boot.trn_boot import _ntff_profile_via_ctypes
        hook = _ntff_profile_via_ctypes("/opt/axon/libaxon_pjrt.so")
        mod = types.ModuleType("antenv.axon_hooks")
        mod.get_axon_ntff_profile_hook = lambda: hook
        mod.set_axon_ntff_profile_hook = lambda h: None
        sys.modules["antenv.axon_hooks"] = mod
    except Exception:
        pass


def kernel(lambdaa, x_re, x_im, y_re, y_im, smaps_re, smaps_im, mask):
    B, C, H, W = 4, 16, N, N
    N_CORES, GROUP_SIZE, N_SAMPLES, CPC, N_ITERS = 8, 4, 2, 4, 10
    lam = float(np.asarray(lambdaa))

    key = (lam, USE_F32R)
    if key not in _CACHE:
        _CACHE.clear()
        _CACHE[key] = build_cg(lam, N_ITERS, CPC, N_SAMPLES, GROUP_SIZE,
                               USE_F32R, N_CORES)
    nc = _CACHE[key]

    fmat = _fmats()
    x_re = np.asarray(x_re, dtype=np.float32)
    x_im = np.asarray(x_im, dtype=np.float32)
    y_re = np.asarray(y_re, dtype=np.float32)
    y_im = np.asarray(y_im, dtype=np.float32)
    smaps_re = np.asarray(smaps_re, dtype=np.float32)
    smaps_im = np.asarray(smaps_im, dtype=np.float32)
    mask = np.asarray(mask, dtype=np.float32)

    in_maps = []
    for core in range(N_CORES):
        g, j = divmod(core, GROUP_SIZE)
        samples = [2 * g, 2 * g + 1]
        coils = list(range(j * CPC, (j + 1) * CPC))
        sm = np.stack([
            np.stack([_complex_tiles(smaps_re[s, c], smaps_im[s, c])
                      for c in coils]) for s in samples])
        yy = np.stack([
            np.stack([_complex_tiles(y_re[s, c], y_im[s, c])
                      for c in coils]) for s in samples])
        mk = np.stack([_to_tiles(mask[s]) for s in samples])
        xi = np.stack([_complex_tiles(x_re[s], x_im[s]) for s in samples])
        in_maps.append({"smaps": sm, "y": yy, "mask": mk, "xin": xi,
                        "fmat": fmat})

    trace = bool(os.environ.get("KERNEL_TRACE"))
    if trace:
        _install_ntff_hook()
    res = run_bass_kernel_spmd(nc, in_maps, core_ids=list(range(N_CORES)),
                               trace=trace)
    global LAST_EXEC_NS
    LAST_EXEC_NS = res.exec_time_ns

    out = np.empty((B, H, W, 2), dtype=np.float32)
    for g in range(2):
        o = res.results[g * GROUP_SIZE]["out"]  # [2, 128, 2, 3, 384]
        for si, s in enumerate((2 * g, 2 * g + 1)):
            out[s, :, :, 0] = _from_tiles(o[si, :, 0])
            out[s, :, :, 1] = _from_tiles(o[si, :, 1])
    return out

